# revision 42
# baseline (speedup 1.0000x reference)
"""Trainium2 Bass kernel for MultiHeadSelfAttention + RoPE (B=2, S=2048, D=1024, H=16).

Sharding: 8 cores = 2 (batch) x 4 (head-groups of 4 heads).

Single fused pipeline per core, all matmul operands in bf16:
  - q/k/v projections chunked by S (proj of chunk c+1 interleaved into the
    attention of chunk c so the PE fills exp-wait slack),
  - RoPE on DVE (bf16 2x/4x modes; pair-swap as four 32-partition-offset
    copies),
  - causal attention with transposed scores (S^T[j,i]) so softmax-exp runs on
    the Scalar engine straight out of PSUM; scores software-pipelined one
    j-tile ahead of attn@V,
  - attn@V against a [v | ones] augmented V (M=128) so the softmax denominator
    lands replicated on psum partitions 64..127 -> per-query normalize is a
    reciprocal + mul on DVE with no broadcast matmul,
  - sliced output projection (per 128-row out-chunk) trailing one chunk behind,
    DMA'd out as it is produced. Host sums the 4 head-group partials per batch.
"""
import math
import os
import sys

import numpy as np

for _p in ("/opt/trn_rl_repo", "/root/.axon_site/_ro/trn_rl_repo"):
    if os.path.isdir(_p) and _p not in sys.path:
        sys.path.insert(0, _p)

import concourse.bacc as bacc
import concourse.bass as bass
import concourse.tile as tile
from concourse import mybir
from concourse import bass_utils

B, S, D = 2, 2048, 1024
H = 16
NCORE = 8
HPC = 4                 # heads per core
E = HPC * 64            # 256: per-core e-width
DK = 64
THETA = 10000.0
CH = 512                # query chunk width
NCH = S // CH           # 4
NJT = S // 128          # 16 j-tiles
F32 = mybir.dt.float32
BF16 = mybir.dt.bfloat16

_programs = {}
LAST_RESULT = None


def _build(share_x: bool, loop_n: int = 0, bench_internal: bool = False):
    nc = bacc.Bacc("TRN2", target_bir_lowering=False)
    kind_in = "Internal" if bench_internal else "ExternalInput"
    xt_qk = nc.dram_tensor("xt_qk", [D, S], BF16, kind=kind_in)
    xt_v = nc.dram_tensor("xt_v", [D, S], BF16, kind=kind_in)
    wqt = nc.dram_tensor("wqt", [D, E], BF16, kind=kind_in)
    wkt = nc.dram_tensor("wkt", [D, E], BF16, kind=kind_in)
    wvt = nc.dram_tensor("wvt", [D, E], BF16, kind=kind_in)
    ot = nc.dram_tensor("ot", [E, D], BF16, kind=kind_in)
    cc = nc.dram_tensor("cc", [128, S], BF16, kind=kind_in)
    ss = nc.dram_tensor("ss", [128, S], BF16, kind=kind_in)
    tri = nc.dram_tensor("tri", [128, 128], BF16, kind=kind_in)
    perm = nc.dram_tensor("perm", [128, 128], BF16, kind=kind_in)
    out_t = nc.dram_tensor(
        "out_t", [D, S], F32, kind="Internal" if bench_internal else "ExternalOutput")
    tick = nc.dram_tensor("tick", [16, 16], F32, kind="ExternalOutput") \
        if bench_internal else None

    Exp = mybir.ActivationFunctionType.Exp
    inv_sqrt_dk = 1.0 / math.sqrt(DK)
    abl = os.environ.get("K_ABL", "")
    eng = os.environ.get("K_ENG", "actevac,actso")
    lookahead = int(os.environ.get("K_LA", "12"))
    rate_mult = float(os.environ.get("K_RM", "1.0"))

    with tile.TileContext(nc) as tc:
        with tc.tile_pool(name="persist", bufs=1) as persist, \
             tc.tile_pool(name="rope", bufs=2) as rope, \
             tc.tile_pool(name="epool", bufs=lookahead + 3) as epool, \
             tc.tile_pool(name="npool", bufs=2) as npool, \
             tc.tile_pool(name="outp", bufs=4) as outp, \
             tc.tile_pool(name="pss", bufs=2, space="PSUM") as pss, \
             tc.tile_pool(name="psu", bufs=2, space="PSUM") as psu, \
             tc.tile_pool(name="psmm", bufs=2, space="PSUM") as psmm:

            x_sb = persist.tile([128, 8, S], BF16, tag="x")
            xv_sb = x_sb if share_x else persist.tile([128, 8, S], BF16, tag="xv")
            wq_sb = persist.tile([128, 8, E], BF16, tag="wq")
            wk_sb = persist.tile([128, 8, E], BF16, tag="wk")
            wv_sb = persist.tile([128, 8, E], BF16, tag="wv")
            ot_sb = persist.tile([128, 2, D], BF16, tag="ot")
            cc_sb = persist.tile([128, S], BF16, tag="cc")
            ss_sb = persist.tile([128, S], BF16, tag="ss")
            tri_sb = persist.tile([128, 2, 128], BF16, tag="tri")
            perm_sb = persist.tile([128, 128], BF16, tag="perm")
            qr_sb = persist.tile([128, 2 * S], BF16, tag="qr")
            kr_sb = persist.tile([128, 2 * S], BF16, tag="kr")
            vaug = persist.tile([128, NJT, 512], BF16, tag="vaug")
            mha_0 = persist.tile([128, S], BF16, tag="mha0")
            mha_1 = persist.tile([128, S], BF16, tag="mha1")
            mha01 = [mha_0, mha_1]
            warm = persist.tile([1, 1], F32, tag="warm")
            if abl:
                # ablation builds read tensors the ablated stage would write
                for t in (qr_sb, kr_sb, mha_0, mha_1):
                    nc.vector.memset(t[:], 0.5)
                nc.vector.memset(vaug[:], 0.5)

            import contextlib
            loop_ctx = tc.For_i(0, loop_n, 1) if loop_n else contextlib.nullcontext()
            with loop_ctx:
                # ---- warm the Exp activation table while DMAs stream ----
                nc.vector.memset(warm[:], 0.0)
                nc.scalar.activation(warm[:], warm[:], Exp, scale=1.0)

                # ---- input DMAs ----
                # weights + tables on the Activation HWDGE queue (idle at t=0)
                nc.scalar.dma_start(wq_sb[:], wqt[:, :].rearrange("(k p) e -> p k e", p=128))
                nc.scalar.dma_start(wk_sb[:], wkt[:, :].rearrange("(k p) e -> p k e", p=128))
                nc.scalar.dma_start(cc_sb[:], cc[:, :])
                nc.scalar.dma_start(ss_sb[:], ss[:, :])
                nc.scalar.dma_start(wv_sb[:], wvt[:, :].rearrange("(k p) e -> p k e", p=128))
                nc.scalar.dma_start(tri_sb[:, 0], tri[:, :])
                nc.scalar.dma_start(tri_sb[:, 1], tri[:, :])
                nc.scalar.dma_start(perm_sb[:], perm[:, :])
                nc.scalar.dma_start(ot_sb[:], ot[:, :].rearrange("(t p) m -> p t m", p=128))
                # x chunk-major on the sync HWDGE queue so proj(0) starts early
                x_re = xt_qk[:, :].rearrange("(k p) s -> p k s", p=128)
                for sc in range(NCH):
                    for kt in range(8):
                        nc.sync.dma_start(x_sb[:, kt, sc * CH:(sc + 1) * CH],
                                          x_re[:, kt, sc * CH:(sc + 1) * CH])
                if not share_x:
                    xv_re = xt_v[:, :].rearrange("(k p) s -> p k s", p=128)
                    for sc in range(NCH):
                        for kt in range(8):
                            nc.sync.dma_start(xv_sb[:, kt, sc * CH:(sc + 1) * CH],
                                              xv_re[:, kt, sc * CH:(sc + 1) * CH])
                # ones columns of the augmented V (value 1.0, exact in bf16)
                ones_view = vaug[:].rearrange("p j (h c) -> p j h c", c=128)[:, :, :, 0:64]
                nc.gpsimd.memset(ones_view, 1.0)

                # ---------------- emission helpers ----------------
                def emit_qk_pair(c, pi):
                    # two projection chains interleaved (hides LdWeights);
                    # rope swap via a PERM matmul on the PE.
                    units = [(et, w) for et in range(2) for w in range(2)][2 * pi:2 * pi + 2]
                    pps = []
                    for et, w in units:
                        pps.append(psmm.tile([128, CH], F32, tag="mm",
                                             name=f"pp{c}{et}{w}"))
                    for kt in range(8):
                        for (et, w), pp in zip(units, pps):
                            w_sb = wq_sb if w == 0 else wk_sb
                            nc.tensor.matmul(
                                pp[:],
                                w_sb[:, kt, et * 128:(et + 1) * 128],
                                x_sb[:, kt, c * CH:(c + 1) * CH],
                                start=(kt == 0), stop=(kt == 7))
                    p_sbs = []
                    for (et, w), pp in zip(units, pps):
                        p_sb = rope.tile([128, CH], BF16, tag="p", name=f"p{c}{et}{w}")
                        if "actevac" in eng:
                            nc.scalar.copy(p_sb[:], pp[:])
                        else:
                            nc.vector.tensor_copy(p_sb[:], pp[:])
                        p_sbs.append(p_sb)
                    psw_pss = []
                    for (et, w), p_sb in zip(units, p_sbs):
                        psw_ps = psmm.tile([128, CH], F32, tag="mm",
                                           name=f"pw{c}{et}{w}")
                        nc.tensor.matmul(psw_ps[:], perm_sb[:], p_sb[:],
                                         start=True, stop=True)
                        psw_pss.append(psw_ps)
                    for (et, w), p_sb, psw_ps in zip(units, p_sbs, psw_pss):
                        dst = qr_sb if w == 0 else kr_sb
                        psw = rope.tile([128, CH], BF16, tag="psw", name=f"w{c}{et}{w}")
                        with nc.allow_low_precision(reason="bf16 rope"):
                            nc.vector.tensor_mul(psw[:], psw_ps[:],
                                                 ss_sb[:, c * CH:(c + 1) * CH])
                        t_sb = rope.tile([128, CH], BF16, tag="t", name=f"t{c}{et}{w}")
                        nc.gpsimd.tensor_mul(t_sb[:], p_sb[:],
                                             cc_sb[:, c * CH:(c + 1) * CH])
                        nc.vector.tensor_add(
                            dst[:, et * S + c * CH: et * S + (c + 1) * CH],
                            t_sb[:], psw[:])

                def emit_v_pair(c, pi):
                    sts = [4 * c + 2 * pi, 4 * c + 2 * pi + 1]
                    pvs = [psmm.tile([128, CH], F32, tag="mm", name=f"pv{st}")
                           for st in sts]
                    for kt in range(8):
                        for st, pv in zip(sts, pvs):
                            nc.tensor.matmul(
                                pv[:, 0:E],
                                xv_sb[:, kt, st * 128:(st + 1) * 128],
                                wv_sb[:, kt],
                                start=(kt == 0), stop=(kt == 7))
                    for st, pv in zip(sts, pvs):
                        dst = vaug[:, st].rearrange("p (h c) -> p h c", c=128)[:, :, 64:128]
                        nc.vector.tensor_copy(
                            dst, pv[:, 0:E].rearrange("p (h c) -> p h c", c=64))

                def proj_units(c):
                    if "noproj" in abl:
                        return []
                    us = [(emit_qk_pair, (c, pi)) for pi in range(2)]
                    us += [(emit_v_pair, (c, pi)) for pi in range(2)]
                    return us

                # attention SEQ
                SEQ = [(c, hp, jt) for c in range(NCH) for hp in range(2)
                       for jt in range(4 * c + 4)]
                s_tiles = {}
                e_tiles = {}
                u_tiles = {}

                def emit_score(i):
                    c, hp, jt = SEQ[i]
                    base = hp * S
                    off = 128 * (jt - 4 * c) if jt >= 4 * c else 0
                    s_ab = pss.tile([128, 2, CH], F32, tag="s", name=f"s{i}")
                    s_tiles[i] = (s_ab, off)
                    j0 = base + jt * 128
                    i0 = base + c * CH
                    nc.tensor.matmul(
                        s_ab[:, 0, off:CH],
                        kr_sb[0:64, j0:j0 + 128],
                        qr_sb[0:64, i0 + off:i0 + CH],
                        start=True, stop=True, tile_position=(0, 0))
                    nc.tensor.matmul(
                        s_ab[:, 1, off:CH],
                        kr_sb[64:128, j0:j0 + 128],
                        qr_sb[64:128, i0 + off:i0 + CH],
                        start=True, stop=True, tile_position=(64, 0))

                def emit_exp(i):
                    c, hp, jt = SEQ[i]
                    s_ab, off = s_tiles[i]
                    e_ab = epool.tile([128, 2, CH], BF16, tag="e", name=f"e{i}")
                    e_tiles[i] = (e_ab, off)
                    with nc.allow_low_precision(reason="bf16 attention weights"):
                        nc.scalar.activation(e_ab[:, :, off:], s_ab[:, :, off:],
                                             Exp, scale=inv_sqrt_dk)
                    s_tiles.pop(i)
                    if jt >= 4 * c:
                        em = e_ab[:, :, off:off + 128]
                        with nc.allow_low_precision(reason="bf16 mask"):
                            nc.vector.tensor_mul(em, em, tri_sb[:])

                def emit_av(i):
                    c, hp, jt = SEQ[i]
                    njt = 4 * c + 4
                    e_ab, off = e_tiles.pop(i)
                    if jt == 0:
                        u_tiles[(c, hp)] = [
                            psu.tile([128, CH], F32, tag="u", name=f"u{c}{hp}{hb}")
                            for hb in range(2)]
                    u_ab = u_tiles[(c, hp)]
                    for hb in range(2):
                        nc.tensor.matmul(
                            u_ab[hb][:, off:CH],
                            vaug[:, jt, (2 * hp + hb) * 128:(2 * hp + hb + 1) * 128],
                            e_ab[:, hb, off:CH],
                            start=(jt == 0), stop=(jt == njt - 1))

                def emit_norm(c, hp):
                    mha_ = mha01[hp]
                    u_ab = u_tiles.pop((c, hp))
                    for hb in range(2):
                        # approx-fast reciprocal requires partition base 0 for
                        # in and out; vaug is [ones | v] so the denominator
                        # lands on psum partitions 0:64
                        rec = npool.tile([64, CH], F32, tag="rec", name=f"r{c}{hp}{hb}")
                        nc.vector.reciprocal_approx_fast(rec[:], u_ab[hb][0:64, :])
                        with nc.allow_low_precision(reason="bf16 mha"):
                            nc.vector.tensor_mul(
                                mha_[hb * 64:(hb + 1) * 64, c * CH:(c + 1) * CH],
                                u_ab[hb][64:128, :], rec[:])

                def emit_oproj_mt(c, mt):
                    po = psmm.tile([128, CH], F32, tag="mm", name=f"po{c}_{mt}")
                    for vt in range(2):
                        nc.tensor.matmul(
                            po[:],
                            ot_sb[:, vt, mt * 128:(mt + 1) * 128],
                            mha01[vt][:, c * CH:(c + 1) * CH],
                            start=(vt == 0), stop=(vt == 1))
                    so = outp.tile([128, CH], F32, tag="so", name=f"so{c}_{mt}")
                    if "actso" in eng and c < 2:
                        nc.scalar.copy(so[:], po[:])
                    else:
                        nc.vector.tensor_copy(so[:], po[:])
                    nc.sync.dma_start(
                        out_t[mt * 128:(mt + 1) * 128, c * CH:(c + 1) * CH],
                        so[:])

                # ---------------- master schedule ----------------
                # Minimal prologue: just what section (0,hp0) needs, so the
                # Act engine starts exp as early as possible. Everything else
                # flows through a global ordered work queue pumped between
                # attention elements, with per-chunk rates that push proj
                # early (PE-bound chunks) and spread oproj into the
                # Act-bound late chunks.
                if "noproj" not in abl:
                    emit_qk_pair(0, 0)       # q/k for et0: all score(0..3)
                                             # needs; everything else is
                                             # pumped so the exp stream
                                             # starts as early as possible

                pending = []
                state = {"ctr": 0.0, "rate": 0.0}

                def pump():
                    state["ctr"] += state["rate"]
                    while state["ctr"] >= 1.0 and pending:
                        state["ctr"] -= 1.0
                        u_fn, u_args = pending.pop(0)
                        u_fn(*u_args)

                def oproj_units(c):
                    if "nooproj" in abl:
                        return []
                    return [(emit_oproj_mt, (c, mt)) for mt in range(8)]

                def stage_chunk_work(c):
                    # pending survives across chunks (global queue); only the
                    # newly-eligible work is appended here.
                    if c == 0:
                        if "noproj" not in abl:
                            # qk(0,1) first: the exp stream reaches (0,hp1)
                            # within the first pump window
                            pending.append((emit_qk_pair, (0, 1)))
                            pending.append((emit_v_pair, (0, 0)))
                            pending.append((emit_v_pair, (0, 1)))
                            pending.extend(proj_units(1))
                        state["rate"] = 2.5 * rate_mult
                    elif c == 1:
                        pending.extend(proj_units(2))
                        state["rate"] = 1.0 * rate_mult
                    elif c == 2:
                        pending.extend(proj_units(3))
                        pending.extend(oproj_units(0))
                        state["rate"] = 0.9 * rate_mult
                    else:
                        pending.extend(oproj_units(1))
                        pending.extend(oproj_units(2))
                        state["rate"] = 0.55 * rate_mult

                # Two decoupled streams over SEQ: stream A (score -> exp,
                # paced by the Act engine) runs up to LOOKAHEAD elements
                # ahead of stream B (attn@V + norm, paced by PE/PSUM), with
                # the exp outputs buffered in the deeper e-ring. This keeps
                # the Act engine busy straight through section boundaries
                # instead of stalling on the av/norm drain.
                LOOKAHEAD = lookahead
                n = len(SEQ)
                noattn = "noattn" in abl
                if not noattn:
                    emit_score(0)
                score_next = 1
                a = 0
                cur_chunk = -1
                score_chunk = 0
                for b, (c, hp, jt) in enumerate(SEQ):
                    if c != cur_chunk:
                        cur_chunk = c
                        stage_chunk_work(c)
                    pump()
                    if not noattn:
                        while a < n and a <= b + LOOKAHEAD:
                            if score_next < n and score_next <= a + 1:
                                if SEQ[score_next][0] != score_chunk:
                                    # the score stream is crossing into the
                                    # next chunk: its proj must be emitted
                                    score_chunk = SEQ[score_next][0]
                                    while pending:
                                        u_fn, u_args = pending.pop(0)
                                        u_fn(*u_args)
                                emit_score(score_next)
                                score_next += 1
                            emit_exp(a)
                            a += 1
                        emit_av(b)
                        if jt == 4 * c + 3:
                            emit_norm(c, hp)
                while pending:
                    u_fn, u_args = pending.pop(0)
                    u_fn(*u_args)
                if "nooproj" not in abl:
                    for mt in range(8):
                        emit_oproj_mt(NCH - 1, mt)
            if tick is not None:
                nc.sync.dma_start(tick[0:1, 0:1], warm[:])
    nc.compile()
    return nc


def _get_program(share_x: bool):
    if share_x not in _programs:
        _programs[share_x] = _build(share_x)
    return _programs[share_x]


def kernel(x, token_positions, q_weight, k_weight, v_weight, o_weight):
    global LAST_RESULT
    import ml_dtypes
    bf16 = ml_dtypes.bfloat16
    x = np.asarray(x, dtype=np.float32)
    pos = np.asarray(token_positions)
    q_weight = np.asarray(q_weight, dtype=np.float32)
    k_weight = np.asarray(k_weight, dtype=np.float32)
    v_weight = np.asarray(v_weight, dtype=np.float32)
    o_weight = np.asarray(o_weight, dtype=np.float32)

    share = bool(np.array_equal(pos, np.arange(S, dtype=pos.dtype)))
    nc = _get_program(share)

    # rope tables in the [4x(evens,odds-swapped)] block layout
    inv = THETA ** (-np.arange(DK // 2, dtype=np.float32) * 2.0 / DK)
    ang = pos.astype(np.float32)[:, None] * inv[None, :]        # (S, 32)
    C = np.cos(ang).T.astype(np.float32)                        # (32, S)
    S_ = np.sin(ang).T.astype(np.float32)
    CC = np.tile(C, (4, 1)).astype(bf16)                        # (128, S)
    SS = np.concatenate([-S_, S_, -S_, S_], axis=0).astype(bf16)
    ii = np.arange(128)
    tri = (ii[:, None] <= ii[None, :]).astype(bf16)
    perm_mat = (ii[:, None] == (ii[None, :] ^ 32)).astype(bf16)

    in_maps = []
    for core in range(NCORE):
        b, hg = divmod(core, 4)
        h0 = HPC * hg
        perm = []
        for h in range(h0, h0 + HPC):
            perm += list(range(64 * h, 64 * h + 64, 2))
            perm += list(range(64 * h + 1, 64 * h + 64, 2))
        xb = x[b]
        xTv = np.ascontiguousarray(xb.T.astype(bf16))
        xTqk = xTv if share else np.ascontiguousarray(xb[pos].T.astype(bf16))
        ecols = slice(64 * h0, 64 * h0 + E)
        in_maps.append({
            "xt_qk": xTqk,
            "xt_v": xTv,
            "wqt": np.ascontiguousarray(q_weight[perm].T.astype(bf16)),
            "wkt": np.ascontiguousarray(k_weight[perm].T.astype(bf16)),
            "wvt": np.ascontiguousarray(v_weight[ecols].T.astype(bf16)),
            "ot": np.ascontiguousarray(o_weight[:, ecols].T.astype(bf16)),
            "cc": CC,
            "ss": SS,
            "tri": tri,
            "perm": perm_mat,
        })

    res = bass_utils.run_bass_kernel_spmd(nc, in_maps, core_ids=list(range(NCORE)))
    LAST_RESULT = res
    out = np.zeros((B, S, D), np.float32)
    for core in range(NCORE):
        out[core // 4] += res.results[core]["out_t"].T
    return out
